# revision 32
# baseline (speedup 1.0000x reference)
"""GCN (PyG GCNConv-style, up to 3 layers) forward on 8 Trainium2 NeuronCores.

Strategy: data-parallel over the 64 graphs (8 graphs per core).  The
message-passing scatter-add is a dense normalized-adjacency matmul.  A is
built on the host in fp8 (e4m3) and shipped in a DoubleRow-friendly
[j2, p, q, i, n'] block layout, so each A matmul contracts K=256 per
instruction at 0.5 cycles/row (2-4x the bf16/fp32r rate) with 4x less
HBM traffic than fp32.  Matmul operand views are native 4D tiles INDEXED
on the q/pair dims — range-sliced pair-dim views mislower on hardware
(verified by micro-test; CoreSim accepts both).  Per layer on device:
    h   = x @ W            (bf16 matmuls, node-major out, copied to fp8)
    x'  = relu(A @ h + b)  (fp8 DoubleRow matmuls, feature-major out,
                            q-outer order so each PSUM bank's relu
                            overlaps the remaining banks' accumulation)
Orientations alternate so no per-layer transposes are needed.  The node
features are pre-gathered from the 500k-row table AND pre-transposed to
feature-major bf16 on the host (multi-index indirect DMA mislowers on
hardware, and host pre-gather also avoids shipping the replicated 256 MB
table to every core), so each graph's x0 arrives with a single dense
DMA.  relu+bias and the h fp8-quantize copies are split between the
Activation engine and the DVE to balance the elementwise load; the mean
pool is folded into the fc1 activations via accum_out.  Numerics:
fp8 per-node noise is averaged down ~sqrt(2048) by the mean-pool head
(~1.6e-3 final relative error vs the fp32 reference, tolerance 2e-2).
"""

import os
import sys

for _p in ("/opt/trn_rl_repo", "/root/.axon_site/_ro/trn_rl_repo"):
    if os.path.isdir(_p) and _p not in sys.path:
        sys.path.insert(0, _p)

import numpy as np

import concourse.bass as bass
import concourse.bacc as bacc
import concourse.mybir as mybir
import concourse.tile as tile
from concourse import bass2jax

G, N, E = 64, 2048, 32768
D = H = 128
O = 2
ALL = 500_000
P = 128
N_CORES = 8
GPC = G // N_CORES          # graphs per core
NCH = N // P                # 128-row chunks per graph (16)
NPAIR = NCH // 2            # DoubleRow chunk pairs per graph (8)

f32 = mybir.dt.float32
bf16 = mybir.dt.bfloat16
f8 = mybir.dt.float8e4
i32 = mybir.dt.int32

F8NP = mybir.dt.np(f8)       # ml_dtypes.float8_e4m3
BF16NP = mybir.dt.np(bf16)   # ml_dtypes.bfloat16

DR = mybir.MatmulPerfMode.DoubleRow


def _build_program(n_layers: int):
    nc = bacc.Bacc("TRN2", target_bir_lowering=False, debug=False,
                   num_devices=N_CORES)

    # features pre-gathered from the 500k table AND pre-transposed to
    # feature-major on the host: row (g*P + d), col n = x0[g][node n, dim d]
    xt16 = nc.dram_tensor("xt16", [GPC * P, N], bf16, kind="ExternalInput")
    # A^T per graph in fp8 DoubleRow layout: row ((g*NPAIR + j2)*P + p),
    # col ((q*2 + i)*512 + n') = A[src=(2*j2+i)*P+p, dst=q*512+n']
    at8 = nc.dram_tensor("at8", [GPC * NPAIR * P, 4 * 2 * 512], f8,
                         kind="ExternalInput")
    wres = nc.dram_tensor("wres", [D, H], bf16, kind="ExternalInput")
    bres = nc.dram_tensor("bres", [H, 1], f32, kind="ExternalInput")
    gw = nc.dram_tensor("gw", [n_layers, H, H], bf16, kind="ExternalInput")
    gb = nc.dram_tensor("gb", [H, n_layers], f32, kind="ExternalInput")
    wfc = nc.dram_tensor("wfc", [H, H], bf16, kind="ExternalInput")
    bfc = nc.dram_tensor("bfc", [H, 1], f32, kind="ExternalInput")
    wlin = nc.dram_tensor("wlin", [H, O], f32, kind="ExternalInput")
    lbb = nc.dram_tensor("lbb", [GPC, O], f32, kind="ExternalInput")
    out_ls = nc.dram_tensor("out_ls", [GPC, O], f32, kind="ExternalOutput")
    out_lg = nc.dram_tensor("out_lg", [GPC, O], f32, kind="ExternalOutput")

    with tile.TileContext(nc) as tc:
        with tc.tile_pool(name="const", bufs=1) as const, \
             tc.tile_pool(name="apool", bufs=2) as apool, \
             tc.tile_pool(name="xpool", bufs=3) as xpool, \
             tc.tile_pool(name="bpool", bufs=2) as bpool, \
             tc.tile_pool(name="hpool", bufs=2) as hpool, \
             tc.tile_pool(name="fpool", bufs=3) as fpool, \
             tc.tile_pool(name="hps", bufs=4, space="PSUM") as hps, \
             tc.tile_pool(name="aps", bufs=1, space="PSUM") as aps:

            # ---- constants: issued on the ACT HWDGE queue so the SP
            # queue starts graph 0's payload DMAs immediately (13 serialized
            # small loads otherwise delay the first A tile by ~7 us) ----
            wres_sb = const.tile([D, H], bf16)
            nc.scalar.dma_start(out=wres_sb[:], in_=wres[:])
            gw_sb = const.tile([H, n_layers * H], bf16)
            for l in range(n_layers):
                nc.scalar.dma_start(out=gw_sb[:, l * H:(l + 1) * H],
                                    in_=gw[l])
            gb_sb = const.tile([H, n_layers], f32)
            nc.scalar.dma_start(out=gb_sb[:], in_=gb[:])
            bres_sb = const.tile([H, 1], f32)
            nc.scalar.dma_start(out=bres_sb[:], in_=bres[:])
            wfc_sb = const.tile([H, H], bf16)
            nc.scalar.dma_start(out=wfc_sb[:], in_=wfc[:])
            bfc_sb = const.tile([H, 1], f32)
            nc.scalar.dma_start(out=bfc_sb[:], in_=bfc[:])
            wlin_sb = const.tile([H, O], f32)
            nc.scalar.dma_start(out=wlin_sb[:], in_=wlin[:])
            lbb_sb = const.tile([GPC, O], f32)
            nc.scalar.dma_start(out=lbb_sb[:], in_=lbb[:])
            macc = const.tile([P, GPC * 4], f32)
            means = const.tile([P, GPC], f32)

            for g in range(GPC):
                # ---- load this graph's pre-gathered feature-major x0 ----
                xT = xpool.tile([P, N], bf16, tag="xT", name="xT")
                nc.sync.dma_start(out=xT[:], in_=xt16[g * P:(g + 1) * P, :])

                # ---- stream this graph's fp8 A^T pair-chunks into SBUF.
                # Native 4D tiles [P, q, i, n']: matmuls INDEX (never slice)
                # the q dim — sliced pair-dim views mislower on hardware. ----
                at_t = []
                for j2 in range(NPAIR):
                    t = apool.tile([P, 4, 2, 512], f8, tag=f"at{j2}",
                                   name=f"at{j2}")
                    r0 = (g * NPAIR + j2) * P
                    nc.sync.dma_start(
                        out=t[:], in_=at8[r0:r0 + P, :].rearrange(
                            "p (q i n) -> p q i n", q=4, i=2))
                    at_t.append(t)

                # ---- residual branch: x1 = relu(wres.T @ xT + bres) ----
                # relu+bias split between ACT (activation) and DVE
                # (tensor_scalar add+max) to balance the elementwise load.
                x1T = bpool.tile([P, N], bf16, tag="x1T", name="x1T")
                for q in range(4):
                    ps_q = aps.tile([P, 512], f32, tag=f"aps{q}", name=f"rps{q}")
                    nc.tensor.matmul(out=ps_q[:], lhsT=wres_sb[:],
                                     rhs=xT[:, q * 512:(q + 1) * 512],
                                     start=True, stop=True)
                    o = x1T[:, q * 512:(q + 1) * 512]
                    if q < 2:
                        nc.scalar.activation(
                            out=o, in_=ps_q[:],
                            func=mybir.ActivationFunctionType.Relu,
                            bias=bres_sb[:])
                    else:
                        nc.vector.tensor_scalar(
                            out=o, in0=ps_q[:], scalar1=bres_sb[:],
                            scalar2=0.0, op0=mybir.AluOpType.add,
                            op1=mybir.AluOpType.max)

                # ---- GCN layers ----
                x_cur = xT
                for l in range(n_layers):
                    # h = x @ W, node-major, quantized to fp8 chunk layout
                    h8 = hpool.tile([P, NPAIR, 2, P], f8, tag="h8",
                                    name="h8")
                    for jj in range(4):
                        ph = hps.tile([P, 512], f32, tag="hps", name="ph")
                        for c in range(4):
                            j = jj * 4 + c
                            nc.tensor.matmul(
                                out=ph[:, c * P:(c + 1) * P],
                                lhsT=x_cur[:, j * P:(j + 1) * P],
                                rhs=gw_sb[:, l * H:(l + 1) * H],
                                start=True, stop=True)
                        h8_dst = h8[:, jj * 2:(jj + 1) * 2]
                        h8_src = ph[:].rearrange("p (a i f) -> p a i f",
                                                 a=2, i=2)
                        if jj % 2 == 0:
                            nc.vector.tensor_copy(out=h8_dst, in_=h8_src)
                        else:
                            nc.scalar.copy(out=h8_dst, in_=h8_src)
                    # x' = relu(A @ h + b): fp8 DoubleRow, K=256/matmul at
                    # 0.5 cycles/row.  q-outer order: bank q finishes its
                    # accumulation early so its relu overlaps the remaining
                    # banks' matmuls and the next layer's h-matmuls never
                    # stall on the act.
                    xn = xpool.tile([P, N], bf16, tag="xT", name="xn")
                    for q in range(4):
                        ps_q = aps.tile([P, 512], f32, tag=f"aps{q}",
                                        name=f"apsl{q}")
                        for j2 in range(NPAIR):
                            nc.tensor.matmul(
                                out=ps_q[:], lhsT=h8[:, j2],
                                rhs=at_t[j2][:, q],
                                start=(j2 == 0), stop=(j2 == NPAIR - 1),
                                perf_mode=DR)
                        o = xn[:, q * 512:(q + 1) * 512]
                        if q < 2:
                            nc.scalar.activation(
                                out=o, in_=ps_q[:],
                                func=mybir.ActivationFunctionType.Relu,
                                bias=gb_sb[:, l:l + 1])
                        else:
                            nc.vector.tensor_scalar(
                                out=o, in0=ps_q[:], scalar1=gb_sb[:, l:l + 1],
                                scalar2=0.0, op0=mybir.AluOpType.add,
                                op1=mybir.AluOpType.max)
                    x_cur = xn

                # ---- fc1: relu((x3 + x1) @ Wfc + b), residual in PSUM ----
                scr = fpool.tile([P, 512], bf16, tag="scr", name="scr", bufs=2)
                for q in range(4):
                    ps_q = aps.tile([P, 512], f32, tag=f"aps{q}", name=f"fps{q}")
                    nc.tensor.matmul(out=ps_q[:], lhsT=wfc_sb[:],
                                     rhs=x_cur[:, q * 512:(q + 1) * 512],
                                     start=True, stop=False)
                    nc.tensor.matmul(out=ps_q[:], lhsT=wfc_sb[:],
                                     rhs=x1T[:, q * 512:(q + 1) * 512],
                                     start=False, stop=True)
                    nc.scalar.activation(
                        out=scr[:], in_=ps_q[:],
                        func=mybir.ActivationFunctionType.Relu, bias=bfc_sb[:],
                        accum_out=macc[:, g * 4 + q:g * 4 + q + 1])

            # ---- head: means -> logits -> log_softmax ----
            nc.vector.tensor_reduce(
                out=means[:], in_=macc[:].rearrange("p (g q) -> p g q", q=4),
                axis=mybir.AxisListType.X, op=mybir.AluOpType.add)
            plt = hps.tile([P, 512], f32, tag="hps", name="pl")
            pl = plt[:GPC, :O]
            nc.tensor.matmul(out=pl, lhsT=means[:], rhs=wlin_sb[:],
                             start=True, stop=True)
            lg_sb = const.tile([GPC, O], f32)
            nc.scalar.activation(out=lg_sb[:], in_=pl,
                                 func=mybir.ActivationFunctionType.Copy,
                                 scale=1.0 / N)
            nc.vector.tensor_add(out=lg_sb[:], in0=lg_sb[:], in1=lbb_sb[:])
            mx = const.tile([GPC, 1], f32)
            nc.vector.tensor_reduce(out=mx[:], in_=lg_sb[:],
                                    axis=mybir.AxisListType.X,
                                    op=mybir.AluOpType.max)
            tt = const.tile([GPC, O], f32)
            nc.vector.tensor_scalar(out=tt[:], in0=lg_sb[:], scalar1=mx[:],
                                    scalar2=None, op0=mybir.AluOpType.subtract)
            ex = const.tile([GPC, O], f32)
            nc.scalar.activation(out=ex[:], in_=tt[:],
                                 func=mybir.ActivationFunctionType.Exp)
            se = const.tile([GPC, 1], f32)
            nc.vector.tensor_reduce(out=se[:], in_=ex[:],
                                    axis=mybir.AxisListType.X,
                                    op=mybir.AluOpType.add)
            lse = const.tile([GPC, 1], f32)
            nc.scalar.activation(out=lse[:], in_=se[:],
                                 func=mybir.ActivationFunctionType.Ln)
            ls_sb = const.tile([GPC, O], f32)
            nc.vector.tensor_scalar(out=ls_sb[:], in0=tt[:], scalar1=lse[:],
                                    scalar2=None, op0=mybir.AluOpType.subtract)
            nc.sync.dma_start(out=out_lg[:], in_=lg_sb[:])
            nc.sync.dma_start(out=out_ls[:], in_=ls_sb[:])

    nc.compile()
    return nc


class _Runner:
    """Compile once, keep the jitted sharded executable for repeat calls."""

    def __init__(self, n_layers: int):
        import jax
        from jax.sharding import Mesh, PartitionSpec
        from jax.experimental.shard_map import shard_map

        self.jax = jax
        nc = _build_program(n_layers)
        self.nc = nc
        bass2jax.install_neuronx_cc_hook()

        in_names, out_names, out_avals, zero_outs = [], [], [], []
        pid_name = nc.partition_id_tensor.name if nc.partition_id_tensor else None
        for alloc in nc.m.functions[0].allocations:
            if not isinstance(alloc, mybir.MemoryLocationSet):
                continue
            name = alloc.memorylocations[0].name
            if alloc.kind == "ExternalInput":
                if name != pid_name:
                    in_names.append(name)
            elif alloc.kind == "ExternalOutput":
                out_names.append(name)
                shape = tuple(alloc.tensor_shape)
                dtype = mybir.dt.np(alloc.dtype)
                out_avals.append(jax.core.ShapedArray(shape, dtype))
                zero_outs.append(np.zeros(shape, dtype))
        self.in_names = list(in_names)
        self.out_names = out_names
        self.zero_outs = zero_outs
        n_params = len(in_names)
        all_names = in_names + out_names + ([pid_name] if pid_name else [])

        def _body(*args):
            operands = list(args)
            if pid_name is not None:
                operands.append(bass2jax.partition_id_tensor())
            return tuple(bass2jax._bass_exec_p.bind(
                *operands,
                out_avals=tuple(out_avals),
                in_names=tuple(all_names),
                out_names=tuple(out_names),
                lowering_input_output_aliases=(),
                sim_require_finite=True,
                sim_require_nnan=True,
                nc=nc,
            ))

        devices = jax.devices()[:N_CORES]
        mesh = Mesh(np.asarray(devices), ("core",))
        self.fn = jax.jit(
            shard_map(_body, mesh=mesh,
                      in_specs=(PartitionSpec("core"),) * (n_params + len(out_names)),
                      out_specs=(PartitionSpec("core"),) * len(out_names),
                      check_rep=False),
            keep_unused=True)

    def run(self, concat_inputs: list[np.ndarray]):
        jax = self.jax
        concat_zeros = [np.zeros((N_CORES * z.shape[0], *z.shape[1:]), z.dtype)
                        for z in self.zero_outs]
        outs = self.fn(*concat_inputs, *concat_zeros)
        jax.block_until_ready(outs)
        return {name: np.asarray(outs[i]) for i, name in enumerate(self.out_names)}


_RUNNERS: dict[int, _Runner] = {}


def _prepare_inputs(all_features, feature_index, edge_index,
                    lin_res_w, lin_res_b, gcn_w, gcn_b,
                    fc1_w, fc1_b, lin_w, lin_b, n_layers):
    """Build the concatenated (over cores, axis 0) device input list."""
    ei = np.asarray(edge_index).astype(np.int32)

    # pre-gather + pre-transpose the node features on the host:
    # xt16_all[g, d, n] = all_features[feature_index[g, n], d] in bf16
    fi = np.asarray(feature_index).astype(np.int64)
    feats = np.asarray(all_features, np.float32)[fi]        # [G, N, D]
    xt16_all = np.ascontiguousarray(
        feats.transpose(0, 2, 1)).astype(BF16NP)            # [G, D, N]

    # A^T per graph in the fp8 DoubleRow [j2, p, q, i, n'] layout.
    # Duplicate (src,dst) cells accumulate in fp32, then round once to fp8.
    at_all = np.zeros((G, NPAIR * P * 4096), F8NP)
    at_u8 = at_all.view(np.uint8)
    loop = np.arange(N, dtype=np.int32)
    for g in range(G):
        src = ei[g, 0]
        dst = ei[g, 1]
        deg = np.bincount(dst, minlength=N).astype(np.float32) + 1.0
        dinv = 1.0 / np.sqrt(deg)
        coef = dinv[src] * dinv[dst]
        src2 = np.concatenate([src, loop])
        dst2 = np.concatenate([dst, loop])
        keys = ((((src2 >> 8) * P + (src2 & 127)) * 4096)
                + ((dst2 >> 9) * 2 + ((src2 >> 7) & 1)) * 512
                + (dst2 & 511)).astype(np.int32)
        vals = np.concatenate([coef, dinv * dinv]).astype(np.float64)
        order = np.argsort(keys, kind="stable")
        ks, vs = keys[order], vals[order]
        first = np.empty(len(ks), bool)
        first[0] = True
        first[1:] = ks[1:] != ks[:-1]
        starts = np.nonzero(first)[0]
        sums = np.add.reduceat(vs, starts).astype(np.float32)
        np.put(at_u8[g], ks[starts], sums.astype(F8NP).view(np.uint8))
    at_all = at_all.reshape(G, NPAIR * P, 4096)

    wres16 = np.asarray(lin_res_w, np.float32).astype(BF16NP)
    gw16 = np.asarray(gcn_w, np.float32)[:n_layers].astype(BF16NP)
    wfc16 = np.asarray(fc1_w, np.float32).astype(BF16NP)
    gbt = np.ascontiguousarray(np.asarray(gcn_b, np.float32)[:n_layers].T)
    bres = np.ascontiguousarray(np.asarray(lin_res_b, np.float32).reshape(H, 1))
    bfc = np.ascontiguousarray(np.asarray(fc1_b, np.float32).reshape(H, 1))
    wlin = np.ascontiguousarray(lin_w, np.float32)
    lbb = np.tile(np.asarray(lin_b, np.float32).reshape(1, O), (GPC, 1))

    per_core = {}
    per_core["xt16"] = [np.ascontiguousarray(
        xt16_all[c * GPC:(c + 1) * GPC]).reshape(GPC * P, N)
        for c in range(N_CORES)]
    per_core["at8"] = [np.ascontiguousarray(
        at_all[c * GPC:(c + 1) * GPC]).reshape(GPC * NPAIR * P, 4096)
        for c in range(N_CORES)]
    for name, arr in [("wres", wres16), ("bres", bres), ("gw", gw16),
                      ("gb", gbt), ("wfc", wfc16), ("bfc", bfc),
                      ("wlin", wlin), ("lbb", lbb)]:
        per_core[name] = [arr] * N_CORES
    return per_core


def kernel(all_features, feature_index, edge_index, action,
           lin_res_w, lin_res_b, gcn_w, gcn_b,
           fc1_w, fc1_b, lin_w, lin_b):
    n_layers = int(action) + 1
    assert 1 <= n_layers <= 3

    if n_layers not in _RUNNERS:
        _RUNNERS[n_layers] = _Runner(n_layers)
    runner = _RUNNERS[n_layers]

    per_core = _prepare_inputs(
        all_features, feature_index, edge_index,
        lin_res_w, lin_res_b, gcn_w, gcn_b, fc1_w, fc1_b, lin_w, lin_b,
        n_layers)

    concat = [np.concatenate(per_core[name], axis=0)
              for name in runner.in_names]
    outs = runner.run(concat)
    ls = outs["out_ls"].reshape(N_CORES, GPC, O).reshape(G, O)
    lg = outs["out_lg"].reshape(N_CORES, GPC, O).reshape(G, O)
    return np.asarray(ls, np.float32), np.asarray(lg, np.float32)


# revision 33
# speedup vs baseline: 1.0084x; 1.0084x over previous
"""GCN (PyG GCNConv-style, up to 3 layers) forward on 8 Trainium2 NeuronCores.

Strategy: data-parallel over the 64 graphs (8 graphs per core).  The
message-passing scatter-add is a dense normalized-adjacency matmul.  A is
built on the host in fp8 (e4m3) and shipped in a DoubleRow-friendly
[j2, p, q, i, n'] block layout, so each A matmul contracts K=256 per
instruction at 0.5 cycles/row (2-4x the bf16/fp32r rate) with 4x less
HBM traffic than fp32.  Matmul operand views are native 4D tiles INDEXED
on the q/pair dims — range-sliced pair-dim views mislower on hardware
(verified by micro-test; CoreSim accepts both).  Per layer on device:
    h   = x @ W            (bf16 matmuls, node-major out, copied to fp8)
    x'  = relu(A @ h + b)  (fp8 DoubleRow matmuls, feature-major out,
                            q-outer order so each PSUM bank's relu
                            overlaps the remaining banks' accumulation)
Orientations alternate so no per-layer transposes are needed.  The node
features are pre-gathered from the 500k-row table AND pre-transposed to
feature-major bf16 on the host (multi-index indirect DMA mislowers on
hardware, and host pre-gather also avoids shipping the replicated 256 MB
table to every core), so each graph's x0 arrives with a single dense
DMA.  relu+bias and the h fp8-quantize copies are split between the
Activation engine and the DVE to balance the elementwise load; the mean
pool is folded into the fc1 activations via accum_out.  Numerics:
fp8 per-node noise is averaged down ~sqrt(2048) by the mean-pool head
(~1.6e-3 final relative error vs the fp32 reference, tolerance 2e-2).
"""

import os
import sys

for _p in ("/opt/trn_rl_repo", "/root/.axon_site/_ro/trn_rl_repo"):
    if os.path.isdir(_p) and _p not in sys.path:
        sys.path.insert(0, _p)

import numpy as np

import concourse.bass as bass
import concourse.bacc as bacc
import concourse.mybir as mybir
import concourse.tile as tile
from concourse import bass2jax

G, N, E = 64, 2048, 32768
D = H = 128
O = 2
ALL = 500_000
P = 128
N_CORES = 8
GPC = G // N_CORES          # graphs per core
NCH = N // P                # 128-row chunks per graph (16)
NPAIR = NCH // 2            # DoubleRow chunk pairs per graph (8)

f32 = mybir.dt.float32
bf16 = mybir.dt.bfloat16
f8 = mybir.dt.float8e4
i32 = mybir.dt.int32

F8NP = mybir.dt.np(f8)       # ml_dtypes.float8_e4m3
BF16NP = mybir.dt.np(bf16)   # ml_dtypes.bfloat16

DR = mybir.MatmulPerfMode.DoubleRow


def _build_program(n_layers: int):
    nc = bacc.Bacc("TRN2", target_bir_lowering=False, debug=False,
                   num_devices=N_CORES)

    # features pre-gathered from the 500k table AND pre-transposed to
    # feature-major on the host: row (g*P + d), col n = x0[g][node n, dim d]
    xt16 = nc.dram_tensor("xt16", [GPC * P, N], bf16, kind="ExternalInput")
    # A^T per graph in fp8 DoubleRow layout: row ((g*NPAIR + j2)*P + p),
    # col ((q*2 + i)*512 + n') = A[src=(2*j2+i)*P+p, dst=q*512+n']
    at8 = nc.dram_tensor("at8", [GPC * NPAIR * P, 4 * 2 * 512], f8,
                         kind="ExternalInput")
    wres = nc.dram_tensor("wres", [D, H], bf16, kind="ExternalInput")
    bres = nc.dram_tensor("bres", [H, 1], f32, kind="ExternalInput")
    gw = nc.dram_tensor("gw", [n_layers, H, H], bf16, kind="ExternalInput")
    gb = nc.dram_tensor("gb", [H, n_layers], f32, kind="ExternalInput")
    wfc = nc.dram_tensor("wfc", [H, H], bf16, kind="ExternalInput")
    bfc = nc.dram_tensor("bfc", [H, 1], f32, kind="ExternalInput")
    wlin = nc.dram_tensor("wlin", [H, O], f32, kind="ExternalInput")
    lbb = nc.dram_tensor("lbb", [GPC, O], f32, kind="ExternalInput")
    out_ls = nc.dram_tensor("out_ls", [GPC, O], f32, kind="ExternalOutput")
    out_lg = nc.dram_tensor("out_lg", [GPC, O], f32, kind="ExternalOutput")

    with tile.TileContext(nc) as tc:
        with tc.tile_pool(name="const", bufs=1) as const, \
             tc.tile_pool(name="apool", bufs=2) as apool, \
             tc.tile_pool(name="xpool", bufs=3) as xpool, \
             tc.tile_pool(name="bpool", bufs=2) as bpool, \
             tc.tile_pool(name="hpool", bufs=2) as hpool, \
             tc.tile_pool(name="fpool", bufs=3) as fpool, \
             tc.tile_pool(name="hps", bufs=4, space="PSUM") as hps, \
             tc.tile_pool(name="aps", bufs=1, space="PSUM") as aps:

            # ---- constants: issued on the ACT HWDGE queue so the SP
            # queue starts graph 0's payload DMAs immediately (13 serialized
            # small loads otherwise delay the first A tile by ~7 us) ----
            wres_sb = const.tile([D, H], bf16)
            nc.scalar.dma_start(out=wres_sb[:], in_=wres[:])
            gw_sb = const.tile([H, n_layers * H], bf16)
            for l in range(n_layers):
                nc.scalar.dma_start(out=gw_sb[:, l * H:(l + 1) * H],
                                    in_=gw[l])
            gb_sb = const.tile([H, n_layers], f32)
            nc.scalar.dma_start(out=gb_sb[:], in_=gb[:])
            bres_sb = const.tile([H, 1], f32)
            nc.scalar.dma_start(out=bres_sb[:], in_=bres[:])
            wfc_sb = const.tile([H, H], bf16)
            nc.scalar.dma_start(out=wfc_sb[:], in_=wfc[:])
            bfc_sb = const.tile([H, 1], f32)
            nc.scalar.dma_start(out=bfc_sb[:], in_=bfc[:])
            wlin_sb = const.tile([H, O], f32)
            nc.scalar.dma_start(out=wlin_sb[:], in_=wlin[:])
            lbb_sb = const.tile([GPC, O], f32)
            nc.scalar.dma_start(out=lbb_sb[:], in_=lbb[:])
            macc = const.tile([P, GPC * 4], f32)
            means = const.tile([P, GPC], f32)

            for g in range(GPC):
                # ---- load this graph's pre-gathered feature-major x0 ----
                xT = xpool.tile([P, N], bf16, tag="xT", name="xT")
                nc.sync.dma_start(out=xT[:], in_=xt16[g * P:(g + 1) * P, :])

                # ---- stream this graph's fp8 A^T pair-chunks into SBUF.
                # Native 4D tiles [P, q, i, n']: matmuls INDEX (never slice)
                # the q dim — sliced pair-dim views mislower on hardware. ----
                at_t = []
                for j2 in range(NPAIR):
                    t = apool.tile([P, 4, 2, 512], f8, tag=f"at{j2}",
                                   name=f"at{j2}")
                    r0 = (g * NPAIR + j2) * P
                    nc.sync.dma_start(
                        out=t[:], in_=at8[r0:r0 + P, :].rearrange(
                            "p (q i n) -> p q i n", q=4, i=2))
                    at_t.append(t)

                # ---- residual branch: x1 = relu(wres.T @ xT + bres) ----
                # relu+bias split between ACT (activation) and DVE
                # (tensor_scalar add+max) to balance the elementwise load.
                x1T = bpool.tile([P, N], bf16, tag="x1T", name="x1T")
                for q in range(4):
                    ps_q = aps.tile([P, 512], f32, tag=f"aps{q}", name=f"rps{q}")
                    nc.tensor.matmul(out=ps_q[:], lhsT=wres_sb[:],
                                     rhs=xT[:, q * 512:(q + 1) * 512],
                                     start=True, stop=True)
                    o = x1T[:, q * 512:(q + 1) * 512]
                    if q < 2:
                        nc.scalar.activation(
                            out=o, in_=ps_q[:],
                            func=mybir.ActivationFunctionType.Relu,
                            bias=bres_sb[:])
                    else:
                        nc.vector.tensor_scalar(
                            out=o, in0=ps_q[:], scalar1=bres_sb[:],
                            scalar2=0.0, op0=mybir.AluOpType.add,
                            op1=mybir.AluOpType.max)

                # ---- GCN layers ----
                x_cur = xT
                for l in range(n_layers):
                    # h = x @ W, node-major, quantized to fp8 chunk layout
                    h8 = hpool.tile([P, NPAIR, 2, P], f8, tag="h8",
                                    name="h8")
                    for jj in range(4):
                        ph = hps.tile([P, 512], f32, tag="hps", name="ph")
                        for c in range(4):
                            j = jj * 4 + c
                            nc.tensor.matmul(
                                out=ph[:, c * P:(c + 1) * P],
                                lhsT=x_cur[:, j * P:(j + 1) * P],
                                rhs=gw_sb[:, l * H:(l + 1) * H],
                                start=True, stop=True)
                        h8_dst = h8[:, jj * 2:(jj + 1) * 2]
                        h8_src = ph[:].rearrange("p (a i f) -> p a i f",
                                                 a=2, i=2)
                        if jj % 2 == 0:
                            nc.vector.tensor_copy(out=h8_dst, in_=h8_src)
                        else:
                            nc.scalar.copy(out=h8_dst, in_=h8_src)
                    # x' = relu(A @ h + b): fp8 DoubleRow, K=256/matmul at
                    # 0.5 cycles/row.  q-outer order: bank q finishes its
                    # accumulation early so its relu overlaps the remaining
                    # banks' matmuls and the next layer's h-matmuls never
                    # stall on the act.
                    xn = xpool.tile([P, N], bf16, tag="xT", name="xn")
                    for q in range(4):
                        ps_q = aps.tile([P, 512], f32, tag=f"aps{q}",
                                        name=f"apsl{q}")
                        for j2 in range(NPAIR):
                            nc.tensor.matmul(
                                out=ps_q[:], lhsT=h8[:, j2],
                                rhs=at_t[j2][:, q],
                                start=(j2 == 0), stop=(j2 == NPAIR - 1),
                                perf_mode=DR)
                        o = xn[:, q * 512:(q + 1) * 512]
                        if q < 2:
                            nc.scalar.activation(
                                out=o, in_=ps_q[:],
                                func=mybir.ActivationFunctionType.Relu,
                                bias=gb_sb[:, l:l + 1])
                        else:
                            nc.vector.tensor_scalar(
                                out=o, in0=ps_q[:], scalar1=gb_sb[:, l:l + 1],
                                scalar2=0.0, op0=mybir.AluOpType.add,
                                op1=mybir.AluOpType.max)
                    x_cur = xn

                # ---- fc1: relu((x3 + x1) @ Wfc + b); sum on DVE first ----
                scr = fpool.tile([P, 512], bf16, tag="scr", name="scr", bufs=2)
                xs = bpool.tile([P, N], bf16, tag="xs", name="xs")
                for q in range(4):
                    nc.vector.tensor_tensor(
                        out=xs[:, q * 512:(q + 1) * 512],
                        in0=x_cur[:, q * 512:(q + 1) * 512],
                        in1=x1T[:, q * 512:(q + 1) * 512],
                        op=mybir.AluOpType.add)
                for q in range(4):
                    ps_q = aps.tile([P, 512], f32, tag=f"aps{q}", name=f"fps{q}")
                    nc.tensor.matmul(out=ps_q[:], lhsT=wfc_sb[:],
                                     rhs=xs[:, q * 512:(q + 1) * 512],
                                     start=True, stop=True)
                    nc.scalar.activation(
                        out=scr[:], in_=ps_q[:],
                        func=mybir.ActivationFunctionType.Relu, bias=bfc_sb[:],
                        accum_out=macc[:, g * 4 + q:g * 4 + q + 1])

            # ---- head: means -> logits -> log_softmax ----
            nc.vector.tensor_reduce(
                out=means[:], in_=macc[:].rearrange("p (g q) -> p g q", q=4),
                axis=mybir.AxisListType.X, op=mybir.AluOpType.add)
            plt = hps.tile([P, 512], f32, tag="hps", name="pl")
            pl = plt[:GPC, :O]
            nc.tensor.matmul(out=pl, lhsT=means[:], rhs=wlin_sb[:],
                             start=True, stop=True)
            lg_sb = const.tile([GPC, O], f32)
            nc.scalar.activation(out=lg_sb[:], in_=pl,
                                 func=mybir.ActivationFunctionType.Copy,
                                 scale=1.0 / N)
            nc.vector.tensor_add(out=lg_sb[:], in0=lg_sb[:], in1=lbb_sb[:])
            mx = const.tile([GPC, 1], f32)
            nc.vector.tensor_reduce(out=mx[:], in_=lg_sb[:],
                                    axis=mybir.AxisListType.X,
                                    op=mybir.AluOpType.max)
            tt = const.tile([GPC, O], f32)
            nc.vector.tensor_scalar(out=tt[:], in0=lg_sb[:], scalar1=mx[:],
                                    scalar2=None, op0=mybir.AluOpType.subtract)
            ex = const.tile([GPC, O], f32)
            nc.scalar.activation(out=ex[:], in_=tt[:],
                                 func=mybir.ActivationFunctionType.Exp)
            se = const.tile([GPC, 1], f32)
            nc.vector.tensor_reduce(out=se[:], in_=ex[:],
                                    axis=mybir.AxisListType.X,
                                    op=mybir.AluOpType.add)
            lse = const.tile([GPC, 1], f32)
            nc.scalar.activation(out=lse[:], in_=se[:],
                                 func=mybir.ActivationFunctionType.Ln)
            ls_sb = const.tile([GPC, O], f32)
            nc.vector.tensor_scalar(out=ls_sb[:], in0=tt[:], scalar1=lse[:],
                                    scalar2=None, op0=mybir.AluOpType.subtract)
            nc.sync.dma_start(out=out_lg[:], in_=lg_sb[:])
            nc.sync.dma_start(out=out_ls[:], in_=ls_sb[:])

    nc.compile()
    return nc


class _Runner:
    """Compile once, keep the jitted sharded executable for repeat calls."""

    def __init__(self, n_layers: int):
        import jax
        from jax.sharding import Mesh, PartitionSpec
        from jax.experimental.shard_map import shard_map

        self.jax = jax
        nc = _build_program(n_layers)
        self.nc = nc
        bass2jax.install_neuronx_cc_hook()

        in_names, out_names, out_avals, zero_outs = [], [], [], []
        pid_name = nc.partition_id_tensor.name if nc.partition_id_tensor else None
        for alloc in nc.m.functions[0].allocations:
            if not isinstance(alloc, mybir.MemoryLocationSet):
                continue
            name = alloc.memorylocations[0].name
            if alloc.kind == "ExternalInput":
                if name != pid_name:
                    in_names.append(name)
            elif alloc.kind == "ExternalOutput":
                out_names.append(name)
                shape = tuple(alloc.tensor_shape)
                dtype = mybir.dt.np(alloc.dtype)
                out_avals.append(jax.core.ShapedArray(shape, dtype))
                zero_outs.append(np.zeros(shape, dtype))
        self.in_names = list(in_names)
        self.out_names = out_names
        self.zero_outs = zero_outs
        n_params = len(in_names)
        all_names = in_names + out_names + ([pid_name] if pid_name else [])

        def _body(*args):
            operands = list(args)
            if pid_name is not None:
                operands.append(bass2jax.partition_id_tensor())
            return tuple(bass2jax._bass_exec_p.bind(
                *operands,
                out_avals=tuple(out_avals),
                in_names=tuple(all_names),
                out_names=tuple(out_names),
                lowering_input_output_aliases=(),
                sim_require_finite=True,
                sim_require_nnan=True,
                nc=nc,
            ))

        devices = jax.devices()[:N_CORES]
        mesh = Mesh(np.asarray(devices), ("core",))
        self.fn = jax.jit(
            shard_map(_body, mesh=mesh,
                      in_specs=(PartitionSpec("core"),) * (n_params + len(out_names)),
                      out_specs=(PartitionSpec("core"),) * len(out_names),
                      check_rep=False),
            keep_unused=True)

    def run(self, concat_inputs: list[np.ndarray]):
        jax = self.jax
        concat_zeros = [np.zeros((N_CORES * z.shape[0], *z.shape[1:]), z.dtype)
                        for z in self.zero_outs]
        outs = self.fn(*concat_inputs, *concat_zeros)
        jax.block_until_ready(outs)
        return {name: np.asarray(outs[i]) for i, name in enumerate(self.out_names)}


_RUNNERS: dict[int, _Runner] = {}


def _prepare_inputs(all_features, feature_index, edge_index,
                    lin_res_w, lin_res_b, gcn_w, gcn_b,
                    fc1_w, fc1_b, lin_w, lin_b, n_layers):
    """Build the concatenated (over cores, axis 0) device input list."""
    ei = np.asarray(edge_index).astype(np.int32)

    # pre-gather + pre-transpose the node features on the host:
    # xt16_all[g, d, n] = all_features[feature_index[g, n], d] in bf16
    fi = np.asarray(feature_index).astype(np.int64)
    feats = np.asarray(all_features, np.float32)[fi]        # [G, N, D]
    xt16_all = np.ascontiguousarray(
        feats.transpose(0, 2, 1)).astype(BF16NP)            # [G, D, N]

    # A^T per graph in the fp8 DoubleRow [j2, p, q, i, n'] layout.
    # Duplicate (src,dst) cells accumulate in fp32, then round once to fp8.
    at_all = np.zeros((G, NPAIR * P * 4096), F8NP)
    at_u8 = at_all.view(np.uint8)
    loop = np.arange(N, dtype=np.int32)
    for g in range(G):
        src = ei[g, 0]
        dst = ei[g, 1]
        deg = np.bincount(dst, minlength=N).astype(np.float32) + 1.0
        dinv = 1.0 / np.sqrt(deg)
        coef = dinv[src] * dinv[dst]
        src2 = np.concatenate([src, loop])
        dst2 = np.concatenate([dst, loop])
        keys = ((((src2 >> 8) * P + (src2 & 127)) * 4096)
                + ((dst2 >> 9) * 2 + ((src2 >> 7) & 1)) * 512
                + (dst2 & 511)).astype(np.int32)
        vals = np.concatenate([coef, dinv * dinv]).astype(np.float64)
        order = np.argsort(keys, kind="stable")
        ks, vs = keys[order], vals[order]
        first = np.empty(len(ks), bool)
        first[0] = True
        first[1:] = ks[1:] != ks[:-1]
        starts = np.nonzero(first)[0]
        sums = np.add.reduceat(vs, starts).astype(np.float32)
        np.put(at_u8[g], ks[starts], sums.astype(F8NP).view(np.uint8))
    at_all = at_all.reshape(G, NPAIR * P, 4096)

    wres16 = np.asarray(lin_res_w, np.float32).astype(BF16NP)
    gw16 = np.asarray(gcn_w, np.float32)[:n_layers].astype(BF16NP)
    wfc16 = np.asarray(fc1_w, np.float32).astype(BF16NP)
    gbt = np.ascontiguousarray(np.asarray(gcn_b, np.float32)[:n_layers].T)
    bres = np.ascontiguousarray(np.asarray(lin_res_b, np.float32).reshape(H, 1))
    bfc = np.ascontiguousarray(np.asarray(fc1_b, np.float32).reshape(H, 1))
    wlin = np.ascontiguousarray(lin_w, np.float32)
    lbb = np.tile(np.asarray(lin_b, np.float32).reshape(1, O), (GPC, 1))

    per_core = {}
    per_core["xt16"] = [np.ascontiguousarray(
        xt16_all[c * GPC:(c + 1) * GPC]).reshape(GPC * P, N)
        for c in range(N_CORES)]
    per_core["at8"] = [np.ascontiguousarray(
        at_all[c * GPC:(c + 1) * GPC]).reshape(GPC * NPAIR * P, 4096)
        for c in range(N_CORES)]
    for name, arr in [("wres", wres16), ("bres", bres), ("gw", gw16),
                      ("gb", gbt), ("wfc", wfc16), ("bfc", bfc),
                      ("wlin", wlin), ("lbb", lbb)]:
        per_core[name] = [arr] * N_CORES
    return per_core


def kernel(all_features, feature_index, edge_index, action,
           lin_res_w, lin_res_b, gcn_w, gcn_b,
           fc1_w, fc1_b, lin_w, lin_b):
    n_layers = int(action) + 1
    assert 1 <= n_layers <= 3

    if n_layers not in _RUNNERS:
        _RUNNERS[n_layers] = _Runner(n_layers)
    runner = _RUNNERS[n_layers]

    per_core = _prepare_inputs(
        all_features, feature_index, edge_index,
        lin_res_w, lin_res_b, gcn_w, gcn_b, fc1_w, fc1_b, lin_w, lin_b,
        n_layers)

    concat = [np.concatenate(per_core[name], axis=0)
              for name in runner.in_names]
    outs = runner.run(concat)
    ls = outs["out_ls"].reshape(N_CORES, GPC, O).reshape(G, O)
    lg = outs["out_lg"].reshape(N_CORES, GPC, O).reshape(G, O)
    return np.asarray(ls, np.float32), np.asarray(lg, np.float32)
